# revision 1
# baseline (speedup 1.0000x reference)
"""Trainium2 Bass kernel for a 2-layer LSTM (B=131072, T=49, H=8) + linear head.

Sharding: pure data parallel over batch across 8 cores. Each core processes
B/8 = 16384 batch elements. Inside a core, batch is organized as 2
"superchunks" of 16 groups x 512 columns; the hidden dim (8) of 16 groups is
stacked on the 128 SBUF partitions so every ACT/DVE op uses all 128 lanes.

Per layer-step, gates are computed with block-diagonal matmuls
(K=128, M=128, N=512) into 4 PSUM banks of one [128, 2048] tile laid out as
[i | f | o | g], so one Sigmoid activation covers i,f,o ([128,1536]) and one
Tanh covers g. The c/h elementwise updates are tensor_tensor ops on
[128, 512] tiles split between the Vector and GpSimd engines.
"""

import numpy as np

B, T, H = 131072, 49, 8
NCORES = 8
BC = B // NCORES          # 16384 per core
G = 16                    # batch groups stacked on partitions (K = G*H = 128)
N = 512                   # matmul free dim / elementwise free dim
SC = BC // (G * N)        # 2 superchunks per core
QT = 13                  # timesteps per staged X quarter-tile
NQ = (T + QT - 1) // QT   # 4 quarter-tiles
GATE_FOR_BANK = [0, 1, 3, 2]  # PSUM bank order [i, f, o, g]; ref order i,f,g,o

_PROGRAM_CACHE = {}

# Matmul operand dtype. fp32 matmuls stream at 4 cycles/row on the PE while
# fp16/bf16 stream at 1; fp16's 11-bit mantissa keeps the end-to-end error at
# ~2e-3 relative (vs 1.3e-2 for bf16) since every operand here is O(1).
MM_DT = "float16"


def _build_program():
    import concourse.bacc as bacc
    import concourse.mybir as mybir
    import concourse.tile as tile

    f32 = mybir.dt.float32
    f16 = getattr(mybir.dt, MM_DT)
    AF = mybir.ActivationFunctionType

    nc = bacc.Bacc("TRN2", target_bir_lowering=False, debug=False)

    xt_d = nc.dram_tensor("xt", [T, BC], f16, kind="ExternalInput")
    wrec0_d = nc.dram_tensor("wrec0", [128, 512], f16, kind="ExternalInput")
    wx0_d = nc.dram_tensor("wx0", [128, 512], f16, kind="ExternalInput")
    wrec1_d = nc.dram_tensor("wrec1", [128, 512], f16, kind="ExternalInput")
    win1_d = nc.dram_tensor("win1", [128, 512], f16, kind="ExternalInput")
    wb1_d = nc.dram_tensor("wb1", [97, 512], f16, kind="ExternalInput")
    whead_d = nc.dram_tensor("whead", [128, 16], f16, kind="ExternalInput")
    headb_d = nc.dram_tensor("headb", [16, 1], f32, kind="ExternalInput")
    y_d = nc.dram_tensor("y", [BC], f32, kind="ExternalOutput")

    with tile.TileContext(nc) as tc:
        with (
            tc.tile_pool(name="w", bufs=1) as wpool,
            tc.tile_pool(name="state", bufs=1) as spool,
            tc.tile_pool(name="xs", bufs=1) as xpool,
            tc.tile_pool(name="work", bufs=2) as work,
        ):
            wrec0 = wpool.tile([128, 512], f16, tag="wrec0")
            nc.sync.dma_start(wrec0[:], wrec0_d[:])
            wx0 = wpool.tile([128, 512], f16, tag="wx0")
            nc.sync.dma_start(wx0[:], wx0_d[:])
            wrec1 = wpool.tile([128, 512], f16, tag="wrec1")
            nc.sync.dma_start(wrec1[:], wrec1_d[:])
            win1 = wpool.tile([128, 512], f16, tag="win1")
            nc.sync.dma_start(win1[:], win1_d[:])
            wb1 = wpool.tile([97, 512], f16, tag="wb1")
            nc.sync.dma_start(wb1[:], wb1_d[:])
            whead = wpool.tile([128, 16], f16, tag="whead")
            nc.sync.dma_start(whead[:], whead_d[:])
            headb = wpool.tile([16, 1], f32, tag="headb")
            nc.sync.dma_start(headb[:], headb_d[:])
            ones = wpool.tile([97, 512], f16, tag="ones")
            nc.gpsimd.memset(ones[:], 1.0)

            # X staging: four quarter-tiles per superchunk, rotated through 2
            # buffers. Partition strip 32q..32q+15 holds the x rows for gate
            # bank q (replicated 4x so the four x-matmuls can run in separate
            # PE row-strips concurrently); row 32q+16 stays 1.0 and carries
            # the layer-0 bias through the K=17 matmul.
            xs = {}
            st = {}
            for sc in range(SC):
                for qt in range(NQ):
                    nt = min(QT, T - QT * qt)
                    x_t = xpool.tile([128, QT * N], f16, tag=f"xs{sc}")
                    nc.gpsimd.memset(x_t[:], 1.0)
                    for tt in range(nt):
                        t = QT * qt + tt
                        xrow = xt_d[t, sc * G * N : (sc + 1) * G * N].rearrange(
                            "(g n) -> g n", g=G
                        )
                        for q in range(4):
                            nc.sync.dma_start(
                                x_t[32 * q : 32 * q + 16, N * tt : N * (tt + 1)],
                                xrow,
                            )
                    xs[sc, qt] = x_t
            for sc in range(SC):
                for nm in ("c1", "c2", "h1", "h2"):
                    dt_nm = f32 if nm.startswith("c") else f16
                    tl = spool.tile([128, 512], dt_nm, tag=f"{nm}_{sc}")
                    nc.gpsimd.memset(tl[:], 0.0)
                    st[nm, sc] = tl

            def layer_step(sc, t, layer):
                h_rec = st["h1", sc] if layer == 0 else st["h2", sc]
                c_t = st["c1", sc] if layer == 0 else st["c2", sc]
                wrec = wrec0 if layer == 0 else wrec1
                # Gates PSUM split into two 2-bank tiles so the i/f half can
                # be consumed (and its banks released) before o/g finishes:
                # ga = [i | f], gb = [o | g].
                ga = psum_gates.tile([128, 1024], f32, tag="ga")
                gb = psum_gates.tile([128, 1024], f32, tag="gb")
                qt, tt = t // QT, t % QT

                def bank(q):
                    return (ga, gb)[q // 2][:, 512 * (q % 2) : 512 * (q % 2 + 1)]

                def small_k(q, stop):
                    # One strip of the concurrent small-K burst: x-matmul
                    # (K=17, carries the layer-0 bias via the ones row) or the
                    # layer-1 bias matmul (K=1 against the ones tile).
                    if layer == 0:
                        nc.tensor.matmul(
                            bank(q),
                            wx0[32 * q : 32 * q + 17, 128 * q : 128 * (q + 1)],
                            xs[sc, qt][32 * q : 32 * q + 17, N * tt : N * (tt + 1)],
                            start=False,
                            stop=stop,
                            tile_position=(32 * q, 0),
                            skip_group_check=True,
                        )
                    else:
                        nc.tensor.matmul(
                            bank(q),
                            wb1[32 * q : 32 * q + 1, 128 * q : 128 * (q + 1)],
                            ones[32 * q : 32 * q + 1, :],
                            start=False,
                            stop=stop,
                            tile_position=(32 * q, 0),
                            skip_group_check=True,
                        )

                # i/f half first so sigma-if (and the c-path) starts early.
                for half in range(2):
                    for q in (2 * half, 2 * half + 1):
                        nc.tensor.matmul(
                            bank(q),
                            wrec[:, 128 * q : 128 * (q + 1)],
                            h_rec[:],
                            start=True,
                            stop=False,
                            skip_group_check=True,
                        )
                        if layer == 1:
                            nc.tensor.matmul(
                                bank(q),
                                win1[:, 128 * q : 128 * (q + 1)],
                                st["h1", sc][:],
                                start=False,
                                stop=False,
                                skip_group_check=True,
                            )
                    for q in (2 * half, 2 * half + 1):
                        small_k(q, stop=True)
                    if half == 0:
                        sif = work.tile([128, 1024], f32, tag=f"sif{sc}")
                        nc.scalar.activation(sif[:], ga[:], AF.Sigmoid)

                gt = work.tile([128, 512], f32, tag=f"gt{sc}")
                nc.scalar.activation(gt[:], gb[:, 512:1024], AF.Tanh)
                so = work.tile([128, 512], f32, tag=f"so{sc}")
                nc.scalar.activation(so[:], gb[:, 0:512], AF.Sigmoid)
                m1 = work.tile([128, 512], f32, tag=f"m1{sc}")
                nc.gpsimd.tensor_mul(m1[:], sif[:, 512:1024], c_t[:])
                m2 = work.tile([128, 512], f32, tag=f"m2{sc}")
                nc.vector.tensor_mul(m2[:], sif[:, 0:512], gt[:])
                nc.vector.tensor_add(c_t[:], m1[:], m2[:])
                th = work.tile([128, 512], f32, tag=f"th{sc}")
                nc.scalar.activation(th[:], c_t[:], AF.Tanh)
                h_out = st["h1", sc] if layer == 0 else st["h2", sc]
                nc.vector.tensor_mul(h_out[:], so[:], th[:])

            with tc.tile_pool(name="psum_gates", bufs=2, space="PSUM") as psum_gates:
                for t in range(T):
                    for sc in range(SC):
                        layer_step(sc, t, 0)
                    for sc in range(SC):
                        layer_step(sc, t, 1)

            with tc.tile_pool(name="psum_head", bufs=2, space="PSUM") as psum_head:
                for sc in range(SC):
                    r2 = work.tile([128, 512], f16, tag="r2")
                    nc.scalar.activation(r2[:], st["h2", sc][:], AF.Relu)
                    hp = psum_head.tile([16, 512], f32, tag="hp")
                    nc.tensor.matmul(
                        hp[:], (whead[:, 0:16]), (r2[:]),
                        start=True, stop=True,
                    )
                    ysb = work.tile([16, 512], f32, tag="ysb")
                    nc.scalar.activation(ysb[:], hp[:], AF.Relu, bias=headb[:])
                    nc.sync.dma_start(
                        y_d[sc * G * N : (sc + 1) * G * N].rearrange("(g n) -> g n", g=G),
                        ysb[:],
                    )

    nc.compile()
    return nc


def _get_program():
    if "nc" not in _PROGRAM_CACHE:
        _PROGRAM_CACHE["nc"] = _build_program()
    return _PROGRAM_CACHE["nc"]


def _pack_weights(W_ih0, W_hh0, b_ih0, b_hh0, W_ih1, W_hh1, b_ih1, b_hh1, W_lin, b_lin):
    b0 = (b_ih0 + b_hh0).astype(np.float32)
    b1 = (b_ih1 + b_hh1).astype(np.float32)
    wrec0 = np.zeros((128, 512), np.float32)
    wx0 = np.zeros((128, 512), np.float32)
    wrec1 = np.zeros((128, 512), np.float32)
    win1 = np.zeros((128, 512), np.float32)
    wb1 = np.zeros((97, 512), np.float32)
    whead = np.zeros((128, 16), np.float32)
    for q in range(4):
        rq = GATE_FOR_BANK[q]
        blk_hh0 = W_hh0[8 * rq : 8 * rq + 8, :]  # [out j, in k]
        blk_hh1 = W_hh1[8 * rq : 8 * rq + 8, :]
        blk_ih1 = W_ih1[8 * rq : 8 * rq + 8, :]
        for g in range(G):
            cols = slice(128 * q + 8 * g, 128 * q + 8 * g + 8)
            rows = slice(8 * g, 8 * g + 8)
            wrec0[rows, cols] = blk_hh0.T  # lhsT[k, j]
            wrec1[rows, cols] = blk_hh1.T
            win1[rows, cols] = blk_ih1.T
            wx0[32 * q + g, cols] = W_ih0[8 * rq : 8 * rq + 8, 0]
            wb1[32 * q, cols] = b1[8 * rq : 8 * rq + 8]
        wx0[32 * q + 16, 128 * q : 128 * (q + 1)] = np.tile(
            b0[8 * rq : 8 * rq + 8], G
        )
    for g in range(G):
        whead[8 * g : 8 * g + 8, g] = W_lin[0, :]
    headb = np.full((16, 1), np.float32(b_lin[0]), np.float32)
    mmdt = np.dtype(MM_DT)
    return {
        "wrec0": wrec0.astype(mmdt),
        "wx0": wx0.astype(mmdt),
        "wrec1": wrec1.astype(mmdt),
        "win1": win1.astype(mmdt),
        "wb1": wb1.astype(mmdt),
        "whead": whead.astype(mmdt),
        "headb": headb,
    }


def _make_in_maps(X, packs):
    Xt = np.ascontiguousarray(np.asarray(X)[:, :, 0].T.astype(np.dtype(MM_DT)))  # [T, B]
    in_maps = []
    for r in range(NCORES):
        m = dict(packs)
        m["xt"] = np.ascontiguousarray(Xt[:, r * BC : (r + 1) * BC])
        in_maps.append(m)
    return in_maps


def kernel(X, W_ih0, W_hh0, b_ih0, b_hh0, W_ih1, W_hh1, b_ih1, b_hh1, W_lin, b_lin,
           _trace=False, _trace_kwargs=None):
    from concourse.bass_utils import run_bass_kernel_spmd

    packs = _pack_weights(
        np.asarray(W_ih0), np.asarray(W_hh0), np.asarray(b_ih0), np.asarray(b_hh0),
        np.asarray(W_ih1), np.asarray(W_hh1), np.asarray(b_ih1), np.asarray(b_hh1),
        np.asarray(W_lin), np.asarray(b_lin),
    )
    nc = _get_program()
    in_maps = _make_in_maps(X, packs)
    res = run_bass_kernel_spmd(
        nc, in_maps, list(range(NCORES)), trace=_trace, **(_trace_kwargs or {})
    )
    y = np.concatenate([res.results[r]["y"] for r in range(NCORES)])
    if _trace:
        return y, res
    return y

